# revision 1
# baseline (speedup 1.0000x reference)
"""Distributed LNO block kernel for 8 TRN2 NeuronCores.

Math (reference):
    phi   = x @ phi_w + phi_b                      [B,N,r]
    psi   = (x @ psi_w + psi_b).reshape(B,N,r,C)   [B,N,r,C]  (never materialized)
    integral_c = einsum('bnrc,bnc->brc', psi, x)/N [B,r,C]
    out   = gelu(x @ W_w + W_b + phi @ integral_c)

Key identity: integral_c[b,r,c] = sum_k psi_w[k, r*C+c] * G[b,k,c] + psi_b[r*C+c] * m[b,c]
with G = (x^T x)/N (per-batch Gram, [B,C,C]) and m = mean_n x.  This removes the
34 GFLOP psi matmul entirely (G costs 0.5 GFLOP).

Distribution: shard N by 8 (each core: 1024 pts of both batches).
  1. per-core G_local/m_local via PE matmuls, AllGather + local sum -> G, m
  2. per-core r-slice (8 r values) of integral via DVE mul + PE ones-matmul
     partition reduction, AllGather -> full integral [B,64,C] everywhere
  3. per-core: w_x + phi @ integral + gelu on its N-shard.
"""

import sys

sys.path.insert(0, "/opt/trn_rl_repo")

import numpy as np

import concourse.bass as bass
import concourse.bacc as bacc
import concourse.mybir as mybir
import concourse.tile as tile
from concourse.bass_utils import run_bass_kernel_spmd

FP = mybir.dt.float32
AF = mybir.ActivationFunctionType

B, N, C, R = 2, 8192, 128, 64
NCORES = 8
NSH = N // NCORES      # 1024 points per core
RL = R // NCORES       # 8 rank rows per core
NT = NSH // 128        # 8 n-tiles of 128 per batch
CP1 = C + 1            # G columns + mean column

_CACHE = {}
LAST_RESULTS = None


def _build(debug=False):
    nc = bacc.Bacc(
        "TRN2", target_bir_lowering=False, debug=False, num_devices=NCORES
    )

    x_in = nc.dram_tensor("x", [B, NSH, C], FP, kind="ExternalInput")
    psiw_in = nc.dram_tensor("psi_w", [C, RL * C], FP, kind="ExternalInput")
    psib_in = nc.dram_tensor("psi_b", [RL, C], FP, kind="ExternalInput")
    phiw_in = nc.dram_tensor("phi_w", [C, R], FP, kind="ExternalInput")
    phib_in = nc.dram_tensor("phi_b", [1, R], FP, kind="ExternalInput")
    ww_in = nc.dram_tensor("W_w", [C, C], FP, kind="ExternalInput")
    wb_in = nc.dram_tensor("W_b", [1, C], FP, kind="ExternalInput")
    id_in = nc.dram_tensor("ident", [128, 128], FP, kind="ExternalInput")
    out_ext = nc.dram_tensor("out", [B, NSH, C], FP, kind="ExternalOutput")
    if debug:
        gdbg_ext = nc.dram_tensor("gdbg", [128, B * CP1], FP, kind="ExternalOutput")
        idbg_ext = nc.dram_tensor("idbg", [R, B * C], FP, kind="ExternalOutput")
        pdbg_ext = nc.dram_tensor("pdbg", [R, B * NSH], FP, kind="ExternalOutput")
        xtdbg_ext = nc.dram_tensor("xtdbg", [128, B * NSH], FP, kind="ExternalOutput")

    with tile.TileContext(nc) as tc:
        with (
            tc.tile_pool(name="big", bufs=1) as bigp,
            tc.tile_pool(name="outs", bufs=4) as outp,
            tc.tile_pool(name="dram", bufs=1, space="DRAM") as dramp,
            tc.tile_pool(name="gmps", bufs=1, space="PSUM") as gmp,
            tc.tile_pool(name="scr", bufs=5, space="PSUM") as scrp,
        ):
            dma = nc.sync.dma_start
            dmae = [
                nc.sync.dma_start,
                nc.scalar.dma_start,
                nc.gpsimd.dma_start,
            ]

            # ---- static SBUF tiles ----
            ones = bigp.tile([128, 1], FP, name="ones")
            id_sb = bigp.tile([128, 128], FP, name="id_sb")
            x_sb = bigp.tile([128, B * NT * 129], FP, name="x_sb")
            xT_sb = bigp.tile([128, B * NSH], FP, name="xT_sb")
            psiw_sb = bigp.tile([128, RL * C], FP, name="psiw_sb")
            psib_st = bigp.tile([RL, C], FP, name="psib_st")
            psibT_sb = bigp.tile([128, RL], FP, name="psibT_sb")
            phiw_sb = bigp.tile([128, R], FP, name="phiw_sb")
            phib_st = bigp.tile([1, R], FP, name="phib_st")
            phibT_sb = bigp.tile([R, 1], FP, name="phibT_sb")
            ww_sb = bigp.tile([128, C], FP, name="ww_sb")
            gloc_sb = bigp.tile([128, B * CP1], FP, name="gloc_sb")
            prod_sb = bigp.tile([128, B * RL * C], FP, name="prod_sb")
            term_sb = bigp.tile([128, B * RL], FP, name="term_sb")
            intT_sb = bigp.tile([128, B * RL], FP, name="intT_sb")
            intRow_sb = bigp.tile([B * RL, C], FP, name="intRow_sb")
            phiT_sb = bigp.tile([R + 1, B * NSH], FP, name="phiT_sb")
            wx_sb = bigp.tile([128, B * NSH], FP, name="wx_sb")
            integ_sb = bigp.tile([R + 1, B * C], FP, name="integ_sb")

            # ---- DRAM bounce buffers for collectives ----
            g_bounce = dramp.tile([B, C, CP1], FP, name="g_bounce")
            g_red = dramp.tile(
                [B, C, CP1], FP, name="g_red", addr_space="Shared"
            )
            i_bounce = dramp.tile([B * RL, C], FP, name="i_bounce")
            i_gath = dramp.tile(
                [NCORES, B * RL, C], FP, name="i_gath", addr_space="Shared"
            )

            # ---- constants & weights ----
            nc.vector.memset(ones[:], 1.0)
            xofs = x_sb[:].rearrange("p (t w) -> p t w", w=129)
            nc.vector.memset(xofs[:, :, 128:129], 1.0)
            nc.vector.memset(phiT_sb[R : R + 1, :], 1.0)
            for b in range(B):
                dma(integ_sb[R : R + 1, b * C : (b + 1) * C], wb_in[:])

            # ---- phase A: load x, Gram matmuls + mean col, x transposes ----
            g_ps = [gmp.tile([128, CP1], FP, name=f"g_ps{b}") for b in range(B)]
            for b in range(B):
                for j in range(NT):
                    t = b * NT + j
                    xt = x_sb[:, t * 129 : t * 129 + 128]
                    xt1 = x_sb[:, t * 129 : t * 129 + 129]
                    dmae[t % len(dmae)](xt, x_in[b, j * 128 : (j + 1) * 128, :])
                    nc.tensor.matmul(
                        g_ps[b][:],
                        xt,
                        xt1,
                        start=(j == 0),
                        stop=(j == NT - 1),
                    )
            # evict G_local and bounce out, start AllGather #1
            for b in range(B):
                nc.vector.tensor_copy(
                    gloc_sb[:, b * CP1 : (b + 1) * CP1], g_ps[b][:]
                )
            for b in range(B):
                dma(g_bounce[b, :64], gloc_sb[:64, b * CP1 : (b + 1) * CP1])
                nc.scalar.dma_start(
                    g_bounce[b, 64:], gloc_sb[64:, b * CP1 : (b + 1) * CP1]
                )
            nc.gpsimd.collective_compute(
                "AllReduce",
                mybir.AluOpType.add,
                replica_groups=[list(range(NCORES))],
                ins=[g_bounce.opt()],
                outs=[g_red.opt()],
            )

            # weights (needed from phase C on — load during AllReduce)
            dma(id_sb[:], id_in[:])
            nc.gpsimd.dma_start(psiw_sb[:], psiw_in[:])
            dma(psib_st[:], psib_in[:])
            dma(phiw_sb[:], phiw_in[:])
            dma(phib_st[:], phib_in[:])
            dma(ww_sb[:], ww_in[:])

            # ---- phase C (overlaps AG#1): transposes, phi, w_x ----
            for b in range(B):
                for j in range(NT):
                    t = b * NT + j
                    xT_ps = scrp.tile([128, 128], FP, tag="scr", name=f"xtp{t}")
                    nc.tensor.transpose(
                        xT_ps[:], x_sb[:, t * 129 : t * 129 + 128], id_sb[:]
                    )
                    nc.any.tensor_copy(xT_sb[:, t * 128 : (t + 1) * 128], xT_ps[:])

            pbT_ps = scrp.tile([128, RL], FP, tag="scr", name="pbT_ps")
            nc.tensor.transpose(pbT_ps[:], psib_st[:], id_sb[:RL, :RL])
            nc.vector.tensor_copy(psibT_sb[:], pbT_ps[:])

            fbT_ps = scrp.tile([R, 1], FP, tag="scr", name="fbT_ps")
            nc.tensor.transpose(fbT_ps[:], phib_st[:], id_sb[:1, :1])
            nc.vector.tensor_copy(phibT_sb[:], fbT_ps[:])

            # phiT[r, n] = phi_w^T @ x^T  (+ phi_b via eviction bias)
            for b in range(B):
                for h in range(2):
                    phiT_ps = scrp.tile([R, 512], FP, tag="scr", name=f"ftp{b}{h}")
                    cols = slice(b * NSH + h * 512, b * NSH + (h + 1) * 512)
                    nc.tensor.matmul(
                        phiT_ps[:], phiw_sb[:], xT_sb[:, cols], start=True, stop=True
                    )
                    nc.vector.tensor_scalar_add(
                        phiT_sb[:R, cols], phiT_ps[:], phibT_sb[:]
                    )

            # w_x: one complete matmul per region, evicted to SBUF
            for b in range(B):
                for j in range(NT):
                    t = b * NT + j
                    wxr_ps = scrp.tile([128, 128], FP, tag="scr", name=f"wxp{t}")
                    nc.tensor.matmul(
                        wxr_ps[:],
                        xT_sb[:, t * 128 : (t + 1) * 128],
                        ww_sb[:],
                        start=True,
                        stop=True,
                    )
                    nc.any.tensor_copy(wx_sb[:, t * 128 : (t + 1) * 128], wxr_ps[:])

            # ---- phase D/E: per-b load reduced G, scale, integral slice ----
            graw_sb = bigp.tile([128, B * CP1], FP, name="graw_sb")
            int_ps = scrp.tile([128, B * RL], FP, tag="scr", name="int_ps")
            for b in range(B):
                dma(graw_sb[:64, b * CP1 : (b + 1) * CP1], g_red[b, :64])
                nc.scalar.dma_start(
                    graw_sb[64:, b * CP1 : (b + 1) * CP1], g_red[b, 64:]
                )
                for rl in range(RL):
                    col = b * RL + rl
                    p = prod_sb[:, col * 128 : (col + 1) * 128]
                    nc.vector.tensor_mul(
                        p,
                        psiw_sb[:, rl * 128 : (rl + 1) * 128],
                        graw_sb[:, b * CP1 : b * CP1 + C],
                    )
                    nc.tensor.matmul(
                        int_ps[:, col : col + 1], p, ones[:], start=True, stop=True
                    )
            for b in range(B):
                nc.vector.tensor_scalar_mul(
                    term_sb[:, b * RL : (b + 1) * RL],
                    psibT_sb[:],
                    graw_sb[:, b * CP1 + C : b * CP1 + CP1],
                )
            nc.vector.tensor_add(intT_sb[:], int_ps[:], term_sb[:])
            nc.vector.tensor_scalar_mul(intT_sb[:], intT_sb[:], 1.0 / N)

            intT2_ps = scrp.tile([B * RL, 128], FP, tag="scr", name="intT2_ps")
            nc.tensor.transpose(intT2_ps[:], intT_sb[:], id_sb[:])
            nc.vector.tensor_copy(intRow_sb[:], intT2_ps[:])
            dma(i_bounce[:], intRow_sb[:])
            nc.gpsimd.collective_compute(
                "AllGather",
                mybir.AluOpType.bypass,
                replica_groups=[list(range(NCORES))],
                ins=[i_bounce.opt()],
                outs=[i_gath.opt()],
            )

            # ---- phase H: load full integral [r=64, c] per batch ----
            for b in range(B):
                dmae[b % 2](
                    integ_sb[:R, b * C : (b + 1) * C],
                    i_gath[:, b * RL : (b + 1) * RL, :],
                )

            if debug:
                dma(gdbg_ext[:], gsum_sb[:])
                dma(idbg_ext[:], integ_sb[:R, :])
                dma(pdbg_ext[:], phiT_sb[:R, :])
                dma(xtdbg_ext[:], xT_sb[:])

            # ---- phase I: kernel_out accumulate + gelu + store ----
            for b in range(B):
                for j in range(NT):
                    t = b * NT + j
                    ko_ps = scrp.tile([128, 128], FP, tag="scr", name=f"kop{t}")
                    nc.tensor.matmul(
                        ko_ps[:],
                        phiT_sb[:, t * 128 : (t + 1) * 128],
                        integ_sb[:, b * C : (b + 1) * C],
                        start=True,
                        stop=True,
                    )
                    ot = outp.tile([128, 128], FP, tag="o", name=f"ot{t}")
                    nc.vector.tensor_add(ot[:], wx_sb[:, t * 128 : (t + 1) * 128], ko_ps[:])
                    og = outp.tile([128, 128], FP, tag="og", name=f"og{t}")
                    nc.scalar.activation(og[:], ot[:], AF.Gelu)
                    dmae[t % 2](out_ext[b, j * 128 : (j + 1) * 128, :], og[:])

    nc.compile()
    return nc


def make_in_maps(inputs):
    x = np.ascontiguousarray(np.asarray(inputs["x"], dtype=np.float32))
    W_w = np.ascontiguousarray(np.asarray(inputs["W_w"], dtype=np.float32))
    W_b = np.ascontiguousarray(
        np.asarray(inputs["W_b"], dtype=np.float32).reshape(1, C)
    )
    phi_w = np.ascontiguousarray(np.asarray(inputs["phi_w"], dtype=np.float32))
    phi_b = np.ascontiguousarray(
        np.asarray(inputs["phi_b"], dtype=np.float32).reshape(1, R)
    )
    psi_w = np.ascontiguousarray(np.asarray(inputs["psi_w"], dtype=np.float32))
    psi_b = np.asarray(inputs["psi_b"], dtype=np.float32)
    ident = np.eye(128, dtype=np.float32)

    in_maps = []
    for i in range(NCORES):
        in_maps.append(
            {
                "x": np.ascontiguousarray(x[:, i * NSH : (i + 1) * NSH, :]),
                "psi_w": np.ascontiguousarray(
                    psi_w[:, i * RL * C : (i + 1) * RL * C]
                ),
                "psi_b": np.ascontiguousarray(
                    psi_b[i * RL * C : (i + 1) * RL * C].reshape(RL, C)
                ),
                "phi_w": phi_w,
                "phi_b": phi_b,
                "W_w": W_w,
                "W_b": W_b,
                "ident": ident,
            }
        )

    return in_maps


def kernel(**inputs):
    global LAST_RESULTS
    if "nc" not in _CACHE:
        _CACHE["nc"] = _build(debug=globals().get("DEBUG", False))
    nc = _CACHE["nc"]
    in_maps = make_in_maps(inputs)
    res = run_bass_kernel_spmd(nc, in_maps, core_ids=list(range(NCORES)))
    LAST_RESULTS = res
    outs = [res.results[i]["out"] for i in range(NCORES)]
    return np.concatenate(outs, axis=1)



# revision 13
# speedup vs baseline: 1.0525x; 1.0525x over previous
"""Distributed LNO block kernel for 8 TRN2 NeuronCores.

Math (reference):
    phi   = x @ phi_w + phi_b                      [B,N,r]
    psi   = (x @ psi_w + psi_b).reshape(B,N,r,C)   [B,N,r,C]  (never materialized)
    integral_c = einsum('bnrc,bnc->brc', psi, x)/N [B,r,C]
    out   = gelu(x @ W_w + W_b + phi @ integral_c)

Key identity: integral_c[b,r,c] = sum_k psi_w[k, r*C+c] * G[b,k,c] + psi_b[r*C+c] * m[b,c]
with G = (x^T x)/N (per-batch Gram, [B,C,C]) and m = mean_n x.  This removes the
34 GFLOP psi matmul entirely.

Distribution: shard N by 8 (each core: 1024 pts of both batches).
  1. per-core G_local (+mean col) via PE matmuls, AllReduce (fp16, 66KB) -> global G
  2. per-core r-slice (8 ranks) of integral via one DVE broadcast-mul + one DVE
     free-axis reduce (uses G's symmetry: G^T = G), AllGather (fp16, 4KB/core)
  3. per-core: w_x and phi @ integral accumulate into the SAME PSUM bank
     (no separate add), gelu straight out of PSUM, batched 256KB stores.

All matmul/DVE traffic is fp16 (inputs are cast on the host; accumulation is
fp32 in PSUM / DVE-reduce); rel-err vs the fp32 reference is ~1e-3.
"""

import sys

sys.path.insert(0, "/opt/trn_rl_repo")

import numpy as np

import concourse.bass as bass
import concourse.bacc as bacc
import concourse.mybir as mybir
import concourse.tile as tile
from concourse.bass_utils import run_bass_kernel_spmd

FP = mybir.dt.float32
HF = mybir.dt.float16
AF = mybir.ActivationFunctionType
ALU = mybir.AluOpType

B, N, C, R = 2, 8192, 128, 64
NCORES = 8
NSH = N // NCORES      # 1024 points per core
RL = R // NCORES       # 8 rank rows per core
NT = NSH // 128        # 8 n-tiles of 128 per batch
CP1 = C + 1            # G columns + mean column

_CACHE = {}
LAST_RESULTS = None


def _build():
    nc = bacc.Bacc(
        "TRN2", target_bir_lowering=False, debug=False, num_devices=NCORES
    )

    x_in = nc.dram_tensor("x", [B, NSH, C], HF, kind="ExternalInput")
    psiw_in = nc.dram_tensor("psi_w", [C, RL * C], HF, kind="ExternalInput")
    psib_in = nc.dram_tensor("psi_b", [RL, C], HF, kind="ExternalInput")
    phiw_in = nc.dram_tensor("phi_w", [C, R], HF, kind="ExternalInput")
    phib_in = nc.dram_tensor("phi_b", [1, R], HF, kind="ExternalInput")
    ww_in = nc.dram_tensor("W_w", [C, C], HF, kind="ExternalInput")
    wb_in = nc.dram_tensor("W_b", [1, C], HF, kind="ExternalInput")
    id_in = nc.dram_tensor("ident", [128, 128], HF, kind="ExternalInput")
    out_ext = nc.dram_tensor("out", [B, NSH, C], FP, kind="ExternalOutput")

    with tile.TileContext(nc) as tc:
        with (
            tc.tile_pool(name="big", bufs=1) as bigp,
            tc.tile_pool(name="outs", bufs=3) as outp,
            tc.tile_pool(name="dram", bufs=1, space="DRAM") as dramp,
            tc.tile_pool(name="gmps", bufs=1, space="PSUM") as gmp,
            tc.tile_pool(name="wrk", bufs=1, space="PSUM") as wrkp,
            tc.tile_pool(name="wrkh", bufs=2, space="PSUM") as wrkhp,
            tc.tile_pool(name="kops", bufs=4, space="PSUM") as kop,
        ):
            # ---- static SBUF tiles ----
            id_hf = bigp.tile([128, 128], HF, name="id_hf")
            x_hf = bigp.tile([128, B * NT * 129], HF, name="x_hf")
            xT_hf = bigp.tile([128, B * NSH], HF, name="xT_hf")
            psiw_hf = bigp.tile([128, RL * C], HF, name="psiw_hf")
            psiwT_hf = bigp.tile([128, RL * C], HF, name="psiwT_hf")
            psib_hf = bigp.tile([RL, C], HF, name="psib_hf")
            psibT_hf = bigp.tile([128, RL], HF, name="psibT_hf")
            phiw_hf = bigp.tile([128, R], HF, name="phiw_hf")
            phib_hf = bigp.tile([1, R], HF, name="phib_hf")
            phibT_hf = bigp.tile([R, 1], FP, name="phibT_hf")
            ww_hf = bigp.tile([128, C], HF, name="ww_hf")
            phiT_hf = bigp.tile([R + 1, B * NSH], HF, name="phiT_hf")
            gloc_hf = bigp.tile([128, B * CP1], HF, name="gloc_hf")
            graw_hf = bigp.tile([128, B * CP1], HF, name="graw_hf")
            prod_hf = bigp.tile([128, B * RL * C], HF, name="prod_hf")
            red_f = bigp.tile([128, B * RL], FP, name="red_f")
            msum_f = bigp.tile([128, B], FP, name="msum_f")
            init_f = bigp.tile([128, B * RL], FP, name="init_f")
            intT_hf = bigp.tile([128, B * RL], HF, name="intT_hf")
            intRow_hf = bigp.tile([B * RL, C], HF, name="intRow_hf")
            integ_hf = bigp.tile([R + 1, B * C], HF, name="integ_hf")

            # ---- DRAM bounce buffers for collectives ----
            g_bounce = dramp.tile([128, B * CP1], HF, name="g_bounce")
            g_red = dramp.tile(
                [128, B * CP1], HF, name="g_red", addr_space="Shared"
            )
            i_bounce = dramp.tile([B * RL, C], HF, name="i_bounce")
            i_gath = dramp.tile(
                [NCORES, B * RL, C], HF, name="i_gath", addr_space="Shared"
            )

            xofs = x_hf[:].rearrange("p (t w) -> p t w", w=129)
            nc.vector.memset(xofs[:, :, 128:129], 1.0)
            nc.vector.memset(phiT_hf[R : R + 1, :], 1.0)

            # ---- weight loads (mostly off the critical path) ----
            nc.gpsimd.dma_start(id_hf[:], id_in[:])
            nc.gpsimd.dma_start(psib_hf[:], psib_in[:])
            nc.gpsimd.dma_start(phib_hf[:], phib_in[:])
            nc.gpsimd.dma_start(phiw_hf[:], phiw_in[:])
            nc.gpsimd.dma_start(ww_hf[:], ww_in[:])
            nc.gpsimd.dma_start(psiw_hf[:], psiw_in[:])
            for b in range(B):
                nc.gpsimd.dma_start(
                    integ_hf[R : R + 1, b * C : (b + 1) * C], wb_in[:]
                )

            # ---- x load: 4 batched DMAs (128KB each) on 2 queues ----
            xdma = [nc.sync.dma_start, nc.scalar.dma_start]
            for b in range(B):
                for h in range(2):
                    g0 = b * NT + h * 4
                    src = x_in[b, h * 512 : (h + 1) * 512, :].rearrange(
                        "(t p) c -> p t c", p=128
                    )
                    xdma[h](xofs[:, g0 : g0 + 4, 0:128], src)

            # ---- Gram matmuls (+ mean col via the ones column) ----
            g_ps = gmp.tile([128, B * CP1], FP, name="g_ps")
            for b in range(B):
                for j in range(NT):
                    t = b * NT + j
                    xt = x_hf[:, t * 129 : t * 129 + 128]
                    xt1 = x_hf[:, t * 129 : t * 129 + 129]
                    nc.tensor.matmul(
                        g_ps[:, b * CP1 : (b + 1) * CP1],
                        xt,
                        xt1,
                        start=(j == 0),
                        stop=(j == NT - 1),
                    )
            nc.vector.tensor_copy(gloc_hf[:], g_ps[:])
            nc.sync.dma_start(g_bounce[:], gloc_hf[:])
            nc.gpsimd.collective_compute(
                "AllReduce",
                ALU.add,
                replica_groups=[list(range(NCORES))],
                ins=[g_bounce.opt()],
                outs=[g_red.opt()],
            )

            # ---- overlaps the AllReduce: transposes + phi ----
            # psib/phib transposes (needed for phi eviction / integral)
            pbT_ps = wrkhp.tile([128, 512], HF, tag="wh", name="pbT_ps")
            nc.tensor.transpose(pbT_ps[:, 0:RL], psib_hf[:], id_hf[:RL, :RL])
            nc.vector.tensor_copy(psibT_hf[:], pbT_ps[:, 0:RL])
            fbT_ps = wrkhp.tile([R, 512], HF, tag="wh", name="fbT_ps")
            nc.tensor.transpose(fbT_ps[:, 0:1], phib_hf[:], id_hf[:1, :1])
            nc.vector.tensor_copy(phibT_hf[:], fbT_ps[:, 0:1])

            # x transposes: 4 per PSUM bank, evicted 512 wide on ACT
            for g in range(4):
                xt_ps = wrkhp.tile([128, 512], HF, tag="wh", name=f"xt_ps{g}")
                for i in range(4):
                    t = g * 4 + i
                    nc.tensor.transpose(
                        xt_ps[:, i * 128 : (i + 1) * 128],
                        x_hf[:, t * 129 : t * 129 + 128],
                        id_hf[:],
                    )
                nc.scalar.activation(
                    xT_hf[:, g * 512 : (g + 1) * 512], xt_ps[:], AF.Copy
                )

            # phiT[r, n] = phi_w^T @ x^T + phi_b (bias on eviction)
            for ch in range(4):
                phi_ps = wrkp.tile([R, 512], FP, tag="w", name=f"phi_ps{ch}")
                cols = slice(ch * 512, (ch + 1) * 512)
                nc.tensor.matmul(
                    phi_ps[:], phiw_hf[:], xT_hf[:, cols], start=True, stop=True
                )
                nc.vector.tensor_scalar_add(
                    phiT_hf[:R, cols], phi_ps[:], phibT_hf[:]
                )

            # psi_w^T blocks (for the c-partition integral layout)
            for g in range(2):
                pw_ps = wrkhp.tile([128, 512], HF, tag="wh", name=f"pw_ps{g}")
                for i in range(4):
                    rl = g * 4 + i
                    nc.tensor.transpose(
                        pw_ps[:, i * 128 : (i + 1) * 128],
                        psiw_hf[:, rl * 128 : (rl + 1) * 128],
                        id_hf[:],
                    )
                nc.vector.tensor_copy(
                    psiwT_hf[:, g * 512 : (g + 1) * 512], pw_ps[:]
                )

            # w_x matmuls need only xT/W_w — issue them now so they run
            # under the collective waits; phi@integral accumulates on top
            # after the AllGather lands.
            ko_banks = []
            for g in range(4):
                ko_ps = kop.tile([128, 512], FP, tag="ko", name=f"ko{g}")
                ko_banks.append(ko_ps)
                for i in range(4):
                    t = g * 4 + i
                    # one accumulation group per 2KB PSUM zero-region (the
                    # whole bank): start only on the bank's first matmul —
                    # start=True clears has_written for the entire region.
                    nc.tensor.matmul(
                        ko_ps[:, i * 128 : (i + 1) * 128],
                        xT_hf[:, t * 128 : (t + 1) * 128],
                        ww_hf[:],
                        start=(i == 0),
                        stop=False,
                    )

            # ---- post-AllReduce: integral r-slice on DVE ----
            nc.sync.dma_start(graw_hf[:], g_red[:])
            gview = graw_hf[:].rearrange("p (b w) -> p b w", w=CP1)
            pw4 = (
                psiwT_hf[:]
                .rearrange("p (rl k) -> p rl k", k=128)
                .unsqueeze(1)
                .broadcast_to([128, B, RL, 128])
            )
            g4 = (
                gview[:, :, 0:128]
                .unsqueeze(2)
                .broadcast_to([128, B, RL, 128])
            )
            prod4 = prod_hf[:].rearrange("p (b rl k) -> p b rl k", rl=RL, k=128)
            nc.vector.tensor_mul(prod4, pw4, g4)
            nc.vector.tensor_reduce(red_f[:], prod4, mybir.AxisListType.X, ALU.add)
            nc.vector.tensor_copy(msum_f[:], gview[:, :, 128])
            for b in range(B):
                nc.vector.tensor_scalar(
                    init_f[:, b * RL : (b + 1) * RL],
                    psibT_hf[:],
                    msum_f[:, b : b + 1],
                    1.0 / N,
                    ALU.mult,
                    ALU.mult,
                )
            nc.vector.tensor_scalar_mul(red_f[:], red_f[:], 1.0 / N)
            with nc.allow_low_precision(reason="fp16 integral intermediate"):
                nc.vector.tensor_add(intT_hf[:], red_f[:], init_f[:])

            intT2_ps = wrkhp.tile([B * RL, 512], HF, tag="wh", name="intT2_ps")
            nc.tensor.transpose(intT2_ps[:, 0:128], intT_hf[:], id_hf[:])
            nc.vector.tensor_copy(intRow_hf[:], intT2_ps[:, 0:128])
            nc.sync.dma_start(i_bounce[:], intRow_hf[:])
            nc.gpsimd.collective_compute(
                "AllGather",
                ALU.bypass,
                replica_groups=[list(range(NCORES))],
                ins=[i_bounce.opt()],
                outs=[i_gath.opt()],
            )

            # ---- post-AllGather: full integral, fused tail ----
            idma = [nc.sync.dma_start, nc.scalar.dma_start]
            for b in range(B):
                idma[b % 2](
                    integ_hf[:R, b * C : (b + 1) * C],
                    i_gath[:, b * RL : (b + 1) * RL, :],
                )

            # per 4-tile group: kernel_out accumulates onto the pre-computed
            # w_x PSUM bank; gelu reads PSUM directly; one 256KB store per
            # group.
            for g in range(4):
                b, h = divmod(g, 2)
                ko_ps = ko_banks[g]
                for i in range(4):
                    t = g * 4 + i
                    nc.tensor.matmul(
                        ko_ps[:, i * 128 : (i + 1) * 128],
                        phiT_hf[:, t * 128 : (t + 1) * 128],
                        integ_hf[:, b * C : (b + 1) * C],
                        start=False,
                        stop=(i == 3),
                    )
                og = outp.tile([128, 512], FP, tag="og", name=f"og{g}")
                nc.scalar.activation(og[:], ko_ps[:], AF.Gelu)
                dst = out_ext[b, h * 512 : (h + 1) * 512, :].rearrange(
                    "(t p) c -> p t c", p=128
                )
                idma[g % 2](dst, og[:].rearrange("p (t c) -> p t c", c=128))

    nc.compile()
    return nc


def make_in_maps(inputs):
    x = np.asarray(inputs["x"], dtype=np.float32).astype(np.float16)
    W_w = np.asarray(inputs["W_w"], dtype=np.float32).astype(np.float16)
    W_b = (
        np.asarray(inputs["W_b"], dtype=np.float32)
        .reshape(1, C)
        .astype(np.float16)
    )
    phi_w = np.asarray(inputs["phi_w"], dtype=np.float32).astype(np.float16)
    phi_b = (
        np.asarray(inputs["phi_b"], dtype=np.float32)
        .reshape(1, R)
        .astype(np.float16)
    )
    psi_w = np.asarray(inputs["psi_w"], dtype=np.float32).astype(np.float16)
    psi_b = np.asarray(inputs["psi_b"], dtype=np.float32).astype(np.float16)
    ident = np.eye(128, dtype=np.float16)

    in_maps = []
    for i in range(NCORES):
        in_maps.append(
            {
                "x": np.ascontiguousarray(x[:, i * NSH : (i + 1) * NSH, :]),
                "psi_w": np.ascontiguousarray(
                    psi_w[:, i * RL * C : (i + 1) * RL * C]
                ),
                "psi_b": np.ascontiguousarray(
                    psi_b[i * RL * C : (i + 1) * RL * C].reshape(RL, C)
                ),
                "phi_w": phi_w,
                "phi_b": phi_b,
                "W_w": W_w,
                "W_b": W_b,
                "ident": ident,
            }
        )

    return in_maps


def kernel(**inputs):
    global LAST_RESULTS
    if "nc" not in _CACHE:
        _CACHE["nc"] = _build()
    nc = _CACHE["nc"]
    in_maps = make_in_maps(inputs)
    res = run_bass_kernel_spmd(nc, in_maps, core_ids=list(range(NCORES)))
    LAST_RESULTS = res
    outs = [res.results[i]["out"] for i in range(NCORES)]
    return np.concatenate(outs, axis=1)


# revision 18
# speedup vs baseline: 1.1075x; 1.0523x over previous
"""Distributed LNO block kernel for 8 TRN2 NeuronCores.

Math (reference):
    phi   = x @ phi_w + phi_b                      [B,N,r]
    psi   = (x @ psi_w + psi_b).reshape(B,N,r,C)   [B,N,r,C]  (never materialized)
    integral_c = einsum('bnrc,bnc->brc', psi, x)/N [B,r,C]
    out   = gelu(x @ W_w + W_b + phi @ integral_c)

Key identity: integral_c[b,r,c] = sum_k psi_w[k, r*C+c] * G[b,k,c] + psi_b[r*C+c] * m[b,c]
with G = (x^T x)/N (per-batch Gram, [B,C,C]) and m = mean_n x.  This removes the
34 GFLOP psi matmul entirely.

Distribution: shard N by 8 (each core: 1024 pts of both batches).
  1. per-core scaled Gram G_loc/N (+mean col via a baked-in ones column) on PE,
     AllReduce (fp16, 66KB) -> global G/N
  2. per-core r-slice (8 ranks) of the integral via one DVE broadcast-mul and
     one DVE free-axis reduce (uses G's symmetry: G^T = G, so no G transpose),
     AllGather (fp16, 4KB/core) -> full integral everywhere
  3. per-core: w_x and phi @ integral accumulate into the SAME PSUM bank (one
     accumulation group per 2KB zero-region), gelu straight out of PSUM,
     batched 256KB stores.

Everything is fp16 on the wire and in the matmuls (fp32 accumulation); inputs
are pre-cast and pre-transposed on the host so every DMA is contiguous and no
x/psi_w transposes run on the device.  rel-err vs the fp32 reference ~5e-4.
"""

import sys

sys.path.insert(0, "/opt/trn_rl_repo")

import numpy as np

import concourse.bass as bass
import concourse.bacc as bacc
import concourse.mybir as mybir
import concourse.tile as tile
from concourse.bass_utils import run_bass_kernel_spmd

FP = mybir.dt.float32
HF = mybir.dt.float16
AF = mybir.ActivationFunctionType
ALU = mybir.AluOpType

B, N, C, R = 2, 8192, 128, 64
NCORES = 8
NSH = N // NCORES      # 1024 points per core
RL = R // NCORES       # 8 rank rows per core
NT = NSH // 128        # 8 n-tiles of 128 per batch
CP1 = C + 1            # G columns + mean column

_CACHE = {}
LAST_RESULTS = None


def _build():
    nc = bacc.Bacc(
        "TRN2", target_bir_lowering=False, debug=False, num_devices=NCORES
    )

    xt_in = nc.dram_tensor("xt", [128, B * NT * 129], HF, kind="ExternalInput")
    xT_in = nc.dram_tensor("xT", [128, B * NSH], HF, kind="ExternalInput")
    psiwT_in = nc.dram_tensor("psiwT", [128, RL * C], HF, kind="ExternalInput")
    psibF_in = nc.dram_tensor("psibF", [R, C], HF, kind="ExternalInput")
    phiw_in = nc.dram_tensor("phi_w", [C, R], HF, kind="ExternalInput")
    phibT_in = nc.dram_tensor("phibT", [R, 1], FP, kind="ExternalInput")
    ww_in = nc.dram_tensor("W_w", [C, C], HF, kind="ExternalInput")
    wb_in = nc.dram_tensor("W_b", [1, C], HF, kind="ExternalInput")
    id_in = nc.dram_tensor("ident", [128, 128], HF, kind="ExternalInput")
    out_ext = nc.dram_tensor("out", [B, NSH, C], FP, kind="ExternalOutput")

    with tile.TileContext(nc) as tc:
        with (
            tc.tile_pool(name="big", bufs=1) as bigp,
            tc.tile_pool(name="outs", bufs=3) as outp,
            tc.tile_pool(name="dram", bufs=1, space="DRAM") as dramp,
            tc.tile_pool(name="gmps", bufs=1, space="PSUM") as gmp,
            tc.tile_pool(name="wrk", bufs=2, space="PSUM") as wrkp,
            tc.tile_pool(name="wrkh", bufs=1, space="PSUM") as wrkhp,
            tc.tile_pool(name="kops", bufs=4, space="PSUM") as kop,
        ):
            # ---- static SBUF tiles ----
            id_hf = bigp.tile([128, 128], HF, name="id_hf")
            x_hf = bigp.tile([128, B * NT * 129], HF, name="x_hf")
            xT_hf = bigp.tile([128, B * NSH], HF, name="xT_hf")
            psiwT_hf = bigp.tile([128, RL * C], HF, name="psiwT_hf")
            psibF_hf = bigp.tile([R, C], HF, name="psibF_hf")
            ones8_hf = bigp.tile([RL, R], HF, name="ones8_hf")
            mg_hf = bigp.tile([RL, B * C], HF, name="mg_hf")
            mtmp_f = bigp.tile([R, C], FP, name="mtmp_f")
            phiw_hf = bigp.tile([128, R], HF, name="phiw_hf")
            phibT_f = bigp.tile([R, 1], FP, name="phibT_f")
            ww_hf = bigp.tile([128, C], HF, name="ww_hf")
            phiT_hf = bigp.tile([R + 1, B * NSH], HF, name="phiT_hf")
            gloc_hf = bigp.tile([128, B * C], HF, name="gloc_hf")
            graw_hf = bigp.tile([128, B * C], HF, name="graw_hf")
            prod_hf = bigp.tile([128, B * RL * C], HF, name="prod_hf")
            red_f = bigp.tile([128, B * RL], FP, name="red_f")
            intT_hf = bigp.tile([128, B * RL + B], HF, name="intT_hf")
            intRow_hf = bigp.tile([B * RL + B, C], HF, name="intRow_hf")
            integ_hf = bigp.tile([R + 1, B * C], HF, name="integ_hf")

            # ---- DRAM bounce buffers for collectives ----
            g_bounce = dramp.tile([128, B * C], HF, name="g_bounce")
            g_red = dramp.tile(
                [128, B * C], HF, name="g_red", addr_space="Shared"
            )
            i_bounce = dramp.tile([B * RL + B, C], HF, name="i_bounce")
            i_gath = dramp.tile(
                [NCORES, B * RL + B, C], HF, name="i_gath", addr_space="Shared"
            )

            nc.vector.memset(phiT_hf[R : R + 1, :], 1.0)
            nc.vector.memset(ones8_hf[:], 1.0)

            # ---- x tiles: 2 contiguous 258KB DMAs (critical path) ----
            half = NT * 129
            nc.sync.dma_start(x_hf[:, 0:half], xt_in[:, 0:half])
            nc.scalar.dma_start(x_hf[:, half : 2 * half], xt_in[:, half:])

            # ---- remaining loads (consumed under the collective waits) ----
            nc.gpsimd.dma_start(id_hf[:], id_in[:])
            nc.gpsimd.dma_start(psibF_hf[:], psibF_in[:])
            nc.gpsimd.dma_start(phibT_f[:], phibT_in[:])
            nc.gpsimd.dma_start(phiw_hf[:], phiw_in[:])
            nc.gpsimd.dma_start(ww_hf[:], ww_in[:])
            for b in range(B):
                nc.gpsimd.dma_start(
                    integ_hf[R : R + 1, b * C : (b + 1) * C], wb_in[:]
                )
            nc.scalar.dma_start(xT_hf[:], xT_in[:])
            nc.gpsimd.dma_start(psiwT_hf[:], psiwT_in[:])

            # ---- Gram matmuls (+ mean col via the baked-in ones column) ----
            g_ps = gmp.tile([128, B * CP1], FP, name="g_ps")
            for b in range(B):
                for j in range(NT):
                    t = b * NT + j
                    xt = x_hf[:, t * 129 : t * 129 + 128]
                    xt1 = x_hf[:, t * 129 : t * 129 + 129]
                    nc.tensor.matmul(
                        g_ps[:, b * CP1 : (b + 1) * CP1],
                        xt,
                        xt1,
                        start=(j == 0),
                        stop=(j == NT - 1),
                    )
            # evict pre-scaled by 1/N so the AllReduce carries G/N directly;
            # only the G block goes on the wire (65536B -> Mesh algorithm),
            # the local mean columns ride the AllGather instead.
            gpview = g_ps[:].rearrange("p (b w) -> p b w", w=CP1)
            nc.vector.tensor_scalar_mul(
                gloc_hf[:].rearrange("p (b w) -> p b w", w=C),
                gpview[:, :, 0:C],
                1.0 / N,
            )
            nc.vector.tensor_scalar_mul(
                intT_hf[:, B * RL : B * RL + B], gpview[:, :, C], 1.0 / N
            )
            nc.sync.dma_start(g_bounce[:], gloc_hf[:])
            nc.gpsimd.collective_compute(
                "AllReduce",
                ALU.add,
                replica_groups=[list(range(NCORES))],
                ins=[g_bounce.opt()],
                outs=[g_red.opt()],
            )

            # ---- overlaps the AllReduce: phi + the w_x half of the tail ----
            for ch in range(4):
                phi_ps = wrkp.tile([R, 512], FP, tag="w", name=f"phi_ps{ch}")
                cols = slice(ch * 512, (ch + 1) * 512)
                nc.tensor.matmul(
                    phi_ps[:], phiw_hf[:], xT_hf[:, cols], start=True, stop=True
                )
                nc.vector.tensor_scalar_add(
                    phiT_hf[:R, cols], phi_ps[:], phibT_f[:]
                )

            # w_x matmuls need only xT/W_w — issue them now so they run under
            # the collective waits; phi@integral accumulates on top after the
            # AllGather lands.  One accumulation group per PSUM bank: start
            # only on the bank's first matmul (start=True clears has_written
            # for the entire 2KB zero-region).
            ko_banks = []
            for g in range(4):
                ko_ps = kop.tile([128, 512], FP, tag="ko", name=f"ko{g}")
                ko_banks.append(ko_ps)
                for i in range(4):
                    t = g * 4 + i
                    nc.tensor.matmul(
                        ko_ps[:, i * 128 : (i + 1) * 128],
                        xT_hf[:, t * 128 : (t + 1) * 128],
                        ww_hf[:],
                        start=(i == 0),
                        stop=False,
                    )

            # ---- post-AllReduce: integral r-slice on DVE ----
            nc.sync.dma_start(graw_hf[:], g_red[:])
            gview = graw_hf[:].rearrange("p (b w) -> p b w", w=C)
            pw4 = (
                psiwT_hf[:]
                .rearrange("p (rl k) -> p rl k", k=128)
                .unsqueeze(1)
                .broadcast_to([128, B, RL, 128])
            )
            g4 = gview.unsqueeze(2).broadcast_to([128, B, RL, 128])
            prod4 = prod_hf[:].rearrange("p (b rl k) -> p b rl k", rl=RL, k=128)
            nc.vector.tensor_mul(prod4, pw4, g4)
            nc.vector.tensor_reduce(red_f[:], prod4, mybir.AxisListType.X, ALU.add)
            nc.vector.tensor_copy(intT_hf[:, 0 : B * RL], red_f[:])

            intT2_ps = wrkhp.tile([B * RL + B, 512], HF, tag="wh", name="intT2_ps")
            nc.tensor.transpose(intT2_ps[:, 0:128], intT_hf[:], id_hf[:])
            nc.vector.tensor_copy(intRow_hf[:], intT2_ps[:, 0:128])
            nc.sync.dma_start(i_bounce[:], intRow_hf[:])
            nc.gpsimd.collective_compute(
                "AllGather",
                ALU.bypass,
                replica_groups=[list(range(NCORES))],
                ins=[i_bounce.opt()],
                outs=[i_gath.opt()],
            )

            # ---- post-AllGather: full integral, fused tail ----
            idma = [nc.sync.dma_start, nc.gpsimd.dma_start]
            for b in range(B):
                idma[b % 2](
                    integ_hf[:R, b * C : (b + 1) * C],
                    i_gath[:, b * RL : (b + 1) * RL, :],
                )
            # global mean/N = sum over cores of the gathered local means;
            # the all-ones [8, 64] stationary both sums over cores and
            # broadcasts the result across 64 partitions in one matmul.
            # Then integ += psi_b * mean (the psi_b bias term).
            nc.sync.dma_start(mg_hf[:], i_gath[:, B * RL : B * RL + B, :])
            mg_ps = wrkp.tile([R, 512], FP, tag="w", name="mg_ps")
            nc.tensor.matmul(
                mg_ps[:, 0 : B * C], ones8_hf[:], mg_hf[:], start=True, stop=True
            )
            with nc.allow_low_precision(reason="fp16 integral bias"):
                for b in range(B):
                    nc.vector.tensor_mul(
                        mtmp_f[:],
                        psibF_hf[:],
                        mg_ps[:, b * C : (b + 1) * C],
                    )
                    nc.vector.tensor_add(
                        integ_hf[:R, b * C : (b + 1) * C],
                        integ_hf[:R, b * C : (b + 1) * C],
                        mtmp_f[:],
                    )

            # per 4-tile group: kernel_out accumulates onto the pre-computed
            # w_x PSUM bank; gelu reads PSUM directly; one 256KB store per
            # group.
            for g in range(4):
                b, h = divmod(g, 2)
                ko_ps = ko_banks[g]
                for i in range(4):
                    t = g * 4 + i
                    nc.tensor.matmul(
                        ko_ps[:, i * 128 : (i + 1) * 128],
                        phiT_hf[:, t * 128 : (t + 1) * 128],
                        integ_hf[:, b * C : (b + 1) * C],
                        start=False,
                        stop=(i == 3),
                    )
                og = outp.tile([128, 512], FP, tag="og", name=f"og{g}")
                nc.scalar.activation(og[:], ko_ps[:], AF.Gelu)
                dst = out_ext[b, h * 512 : (h + 1) * 512, :].rearrange(
                    "(t p) c -> p t c", p=128
                )
                idma[g % 2](dst, og[:].rearrange("p (t c) -> p t c", c=128))

    nc.compile()
    return nc


def make_in_maps(inputs):
    x = np.asarray(inputs["x"], dtype=np.float32).astype(np.float16)
    W_w = np.asarray(inputs["W_w"], dtype=np.float32).astype(np.float16)
    W_b = (
        np.asarray(inputs["W_b"], dtype=np.float32)
        .reshape(1, C)
        .astype(np.float16)
    )
    phi_w = np.asarray(inputs["phi_w"], dtype=np.float32).astype(np.float16)
    phibT = np.ascontiguousarray(
        np.asarray(inputs["phi_b"], dtype=np.float32).reshape(R, 1)
    )
    psi_w = np.asarray(inputs["psi_w"], dtype=np.float32).astype(np.float16)
    psi_b = np.asarray(inputs["psi_b"], dtype=np.float32).astype(np.float16)
    psibF = np.ascontiguousarray(psi_b.reshape(R, C))
    ident = np.eye(128, dtype=np.float16)

    in_maps = []
    for i in range(NCORES):
        xs = x[:, i * NSH : (i + 1) * NSH, :]          # [B, NSH, C]
        xs_r = xs.reshape(B, NT, 128, C)
        xt = np.ones((128, B * NT, 129), np.float16)
        xt[:, :, :C] = xs_r.transpose(2, 0, 1, 3).reshape(128, B * NT, C)
        xT = xs.transpose(2, 0, 1).reshape(C, B * NSH)
        pw = psi_w[:, i * RL * C : (i + 1) * RL * C]
        psiwT = (
            pw.reshape(C, RL, C).transpose(2, 1, 0).reshape(C, RL * C)
        )

        in_maps.append(
            {
                "xt": np.ascontiguousarray(xt.reshape(128, B * NT * 129)),
                "xT": np.ascontiguousarray(xT),
                "psiwT": np.ascontiguousarray(psiwT),
                "psibF": psibF,
                "phi_w": phi_w,
                "phibT": phibT,
                "W_w": W_w,
                "W_b": W_b,
                "ident": ident,
            }
        )

    return in_maps


def kernel(**inputs):
    global LAST_RESULTS
    if "nc" not in _CACHE:
        _CACHE["nc"] = _build()
    nc = _CACHE["nc"]
    in_maps = make_in_maps(inputs)
    res = run_bass_kernel_spmd(nc, in_maps, core_ids=list(range(NCORES)))
    LAST_RESULTS = res
    outs = [res.results[i]["out"] for i in range(NCORES)]
    return np.concatenate(outs, axis=1)
